# revision 1
# baseline (speedup 1.0000x reference)
"""Trainium2 Bass kernel for causal linear attention (elu+1 feature map) with
output projection + sigmoid gate residual mixing.

Reference computation (B=2, S=1024, D=512, H=8, hd=64):
    q = fmap(x@Wq), k = fmap(x@Wk), v = x@Wv          (fmap = elu+1)
    attn[s] = q[s] . cumsum_t<=s(k[t] v[t]^T) / (q[s] . cumsum(k) + 1e-6)
    out = attn@Wo + bo
    gate = sigmoid([x, out]@Wg + bg)
    y = x + gate*(out - x)

Sharding: 8 cores = (b in {0,1}) x (s-quarter j in {0..3}).  Core (b,j) owns
rows [256j, 256j+256) of batch b.  The causal prefix state (sum over earlier
rows of k^T [v|1]) is recomputed locally from a zero-padded prefix input
(uniform SPMD instruction stream; a mask column keeps padding out of the
state).  No cross-core communication.

Layouts: activations are kept feature-major ("fm", [d, s]) for contractions
over features and row-major for contractions over time + final I/O.  The host
supplies pre-transposed copies of x (layout prep during sharding), and
pre-rounds every tensor consumed by a float32r matmul (f32r = RNE to 11
mantissa bits, verified against the HW cast-DMA) so all loads are plain
HWDGE DMAs of the final bits.
"""

import os
import functools
import numpy as np

B, S, D = 2, 1024, 512
H, HD = 8, 64
SQ = 256          # rows owned per core
PRE = 3 * SQ      # padded prefix rows
NCORE = 8
P = 128

LAST_EXEC_NS = [None]


@functools.lru_cache(maxsize=1)
def _build():
    import concourse.bass as bass
    import concourse.mybir as mybir
    import concourse.tile as tile
    from concourse import bacc

    f32 = mybir.dt.float32
    f32r = mybir.dt.float32r

    nc = bacc.Bacc(
        "TRN2", target_bir_lowering=False, debug=False, num_devices=NCORE
    )

    dx_own = nc.dram_tensor("x_own", [SQ, D], f32, kind="ExternalInput").ap()
    dx_ownT = nc.dram_tensor("x_ownT", [D, SQ], f32r, kind="ExternalInput").ap()
    dx_preT = nc.dram_tensor("x_preT", [D, PRE], f32r, kind="ExternalInput").ap()
    dmask8 = nc.dram_tensor("mask8", [PRE, H], f32, kind="ExternalInput").ap()
    dwq = nc.dram_tensor("Wq", [D, D], f32r, kind="ExternalInput").ap()
    dwk = nc.dram_tensor("Wk", [D, D], f32r, kind="ExternalInput").ap()
    dwv = nc.dram_tensor("Wv", [D, D], f32r, kind="ExternalInput").ap()
    dwo = nc.dram_tensor("Wo", [D, D], f32r, kind="ExternalInput").ap()
    dbo = nc.dram_tensor("bo", [D], f32r, kind="ExternalInput").ap()
    dwg = nc.dram_tensor("Wg", [2 * D, D], f32r, kind="ExternalInput").ap()
    dbg = nc.dram_tensor("bg", [D], f32r, kind="ExternalInput").ap()
    dy = nc.dram_tensor("y", [SQ, D], f32, kind="ExternalOutput").ap()

    with tile.TileContext(nc) as tc:
        _emit(nc, tc, mybir, dx_own, dx_ownT, dx_preT, dmask8, dwq, dwk, dwv,
              dwo, dbo, dwg, dbg, dy)

    nc.compile()
    return nc


def _emit(nc, tc, mybir, dx_own, dx_ownT, dx_preT, dmask8, dwq, dwk, dwv, dwo,
          dbo, dwg, dbg, dy):
    f32 = mybir.dt.float32
    f32r = mybir.dt.float32r
    bf16 = mybir.dt.bfloat16
    AF = mybir.ActivationFunctionType
    OP = mybir.AluOpType
    NPRE = PRE // P           # 6 prefix chunks
    NCH = SQ // P             # 2 own chunks

    import contextlib
    import bass_rust as _br

    def chain(mms):
        # Accumulating matmuls into one PSUM bank must execute in emission
        # order (start=True first, stop=True last) — the Tile scheduler is
        # otherwise free to reorder same-engine instructions.
        for later, earlier in zip(mms[1:], mms[:-1]):
            _br.add_dep_helper(later.ins, earlier.ins, sync=False,
                               reason="psum accumulation order")

    ctx = contextlib.ExitStack()
    with ctx:
        consts = ctx.enter_context(tc.tile_pool(name="consts", bufs=1))
        fmtmp = ctx.enter_context(tc.tile_pool(name="fmtmp", bufs=4))
        prework = ctx.enter_context(tc.tile_pool(name="prework", bufs=4))
        attnwork = ctx.enter_context(tc.tile_pool(name="attnwork", bufs=4))
        outwork = ctx.enter_context(tc.tile_pool(name="outwork", bufs=3))
        # PSUM pools: total concurrent slots must stay <= 8 banks
        pp = ctx.enter_context(tc.tile_pool(name="pp", bufs=3, space="PSUM"))
        pA = ctx.enter_context(tc.tile_pool(name="pA", bufs=3, space="PSUM"))
        pn = ctx.enter_context(tc.tile_pool(name="pn", bufs=2, space="PSUM"))

        # ---------------- constant / persistent loads ----------------
        # Split into per-d-tile / per-chunk DMAs with separate tiles so the
        # prefix pipeline starts as soon as its first operands land, and the
        # transfers spread over multiple DMA queues.
        # DMA emission order == first-use order so the (mostly serial) input
        # stream feeds the compute pipeline just in time.
        dwkr = dwk.rearrange("(t p) e -> p t e", p=P)
        dwvr = dwv.rearrange("(t p) e -> p t e", p=P)
        dwqr = dwq.rearrange("(t p) e -> p t e", p=P)
        dwgr = dwg.rearrange("(t p) e -> p t e", p=P)
        dxpr = dx_preT.rearrange("(t p) s -> p t s", p=P)

        xT_pre_t = [consts.tile([P, 4, P], f32r, tag=f"xpre{c}",
                                name=f"xpre{c}") for c in range(NPRE)]
        nc.sync.dma_start(out=xT_pre_t[0], in_=dxpr[:, :, 0:P])
        wk_t, wv_t, wq_t = [], [], []
        for dt in range(4):
            t = consts.tile([P, D], f32r, tag=f"wk{dt}")
            nc.sync.dma_start(out=t, in_=dwkr[:, dt, :])
            wk_t.append(t)
        for dt in range(4):
            t = consts.tile([P, D], f32r, tag=f"wv{dt}")
            nc.sync.dma_start(out=t, in_=dwvr[:, dt, :])
            wv_t.append(t)
        for c in range(1, NPRE):
            nc.sync.dma_start(out=xT_pre_t[c], in_=dxpr[:, :, P * c:P * c + P])
        m8 = consts.tile([P, NPRE, H], f32)
        nc.sync.dma_start(out=m8,
                          in_=dmask8.rearrange("(c p) h -> p c h", p=P))
        xT_own = consts.tile([P, 4, SQ], f32r)
        nc.sync.dma_start(out=xT_own,
                          in_=dx_ownT.rearrange("(t p) s -> p t s", p=P))
        for dt in range(4):
            t = consts.tile([P, D], f32r, tag=f"wq{dt}")
            nc.sync.dma_start(out=t, in_=dwqr[:, dt, :])
            wq_t.append(t)
        # Wo in [d_local, head, e] layout so per-head K=64 contractions line up
        # with base-0 attn tiles
        wo64 = consts.tile([HD, H, D], f32r)
        nc.sync.dma_start(out=wo64, in_=dwo.rearrange("(h d) e -> d h e", d=HD))
        wg_t = []
        for kt in range(8):
            t = consts.tile([P, D], f32r, tag=f"wg{kt}")
            nc.sync.dma_start(out=t, in_=dwgr[:, kt, :])
            wg_t.append(t)
        bo_row = consts.tile([1, D], f32r)
        nc.sync.dma_start(out=bo_row, in_=dbo.rearrange("(o e) -> o e", o=1))
        bg_row = consts.tile([1, D], f32r)
        nc.sync.dma_start(out=bg_row, in_=dbg.rearrange("(o e) -> o e", o=1))
        bo_fm = consts.tile([P, 4], f32r)
        nc.sync.dma_start(out=bo_fm, in_=dbo.rearrange("(t p) -> p t", p=P))

        ones1_f = consts.tile([1, P], f32)
        nc.vector.memset(ones1_f, 1.0)
        ones1 = consts.tile([1, P], f32r)
        nc.vector.tensor_copy(out=ones1, in_=ones1_f)
        onesP_f = consts.tile([P, NCH * H], f32)
        nc.vector.memset(onesP_f, 1.0)

        # combined causal masks: cols 0:256 = [t <= s] for the t0 block,
        # cols 256:512 = [t+128 <= s] for the t1 block
        maskc = consts.tile([P, 2 * SQ], f32)
        nc.gpsimd.memset(maskc, 0.0)
        nc.gpsimd.affine_select(
            out=maskc[:, 0:SQ], in_=maskc[:, 0:SQ], compare_op=OP.is_gt,
            fill=1.0, base=0, pattern=[[-1, SQ]], channel_multiplier=1)
        nc.gpsimd.affine_select(
            out=maskc[:, SQ:], in_=maskc[:, SQ:], compare_op=OP.is_gt,
            fill=1.0, base=P, pattern=[[-1, SQ]], channel_multiplier=1)

        x_rm = consts.tile([P, NCH, D], f32)
        nc.sync.dma_start(out=x_rm, in_=dx_own.rearrange("(c p) e -> p c e", p=P))

        def fmap_from_psum(psum_ap, out_ap, width, pool, eng=None):
            """out = min(exp(t),1) + relu(t) elementwise from a PSUM tile."""
            e_t = pool.tile([P, width], f32, tag="fm_e")
            nc.scalar.activation(out=e_t, in_=psum_ap, func=AF.Exp)
            r_t = pool.tile([P, width], f32, tag="fm_r")
            nc.scalar.activation(out=r_t, in_=psum_ap, func=AF.Relu)
            (eng or nc.vector).scalar_tensor_tensor(
                out=out_ap, in0=e_t, scalar=1.0, in1=r_t,
                op0=OP.min, op1=OP.add)

        # ---------------- prefix state ----------------
        # state[64*(h%2):+64, h//2, :] accumulates K_h^T [V_h | mask] over all
        # prefix chunks.
        # full-bank shape (512 f32/partition) so partition-offset matmul
        # output slices stay inside one PSUM bank
        state_psum = pn.tile([P, 4, P], f32, tag="pn")
        state_mms = []
        k_rm_t, v_pre_t = {}, {}

        def emit_prefix_proj(c):
            ps_k = pp.tile([P, D], f32, tag="pp")
            chain([nc.tensor.matmul(
                ps_k, lhsT=xT_pre_t[c][:, dt, :],
                rhs=wk_t[dt],
                start=(dt == 0), stop=(dt == 3)) for dt in range(4)])
            k_rm = prework.tile([P, D], bf16, tag="k_rm")
            fmap_from_psum(ps_k, k_rm, D, prework)
            k_rm_t[c] = k_rm

            ps_v = pp.tile([P, D], f32, tag="pp")
            chain([nc.tensor.matmul(
                ps_v, lhsT=xT_pre_t[c][:, dt, :],
                rhs=wv_t[dt],
                start=(dt == 0), stop=(dt == 3)) for dt in range(4)])
            v_pre = prework.tile([P, H, HD + 1], bf16, tag="v_pre")
            nc.vector.tensor_copy(
                out=v_pre[:, :, 0:HD],
                in_=ps_v.rearrange("p (h e) -> p h e", h=H))
            nc.vector.tensor_copy(
                out=v_pre[:, :, HD:HD + 1],
                in_=m8[:, c, :].rearrange("p (h o) -> p h o", o=1))
            v_pre_t[c] = v_pre

        def emit_state(c):
            k_rm, v_pre = k_rm_t.pop(c), v_pre_t.pop(c)
            for h in range(H):
                r, p2 = h % 2, h // 2
                # one accumulation group per 64-partition half of the bank:
                # the start's pending-zero covers only the partitions it
                # touches, so each r-group needs its own start/stop
                state_mms.append(nc.tensor.matmul(
                    state_psum[64 * r:64 * r + 64, p2, 0:HD + 1],
                    lhsT=k_rm[:, HD * h:HD * h + HD],
                    rhs=v_pre[:, h, :],
                    start=(c == 0 and h == r),
                    stop=(c == NPRE - 1 and h == H - 2 + r),
                    tile_position=(0, 64 * r),
                    skip_group_check=True))

        emit_prefix_proj(0)
        for c in range(1, NPRE):
            emit_prefix_proj(c)
            emit_state(c - 1)
        emit_state(NPRE - 1)
        chain(state_mms)

        state_sb = consts.tile([P, 4, HD + 1], f32r)
        nc.vector.tensor_copy(out=state_sb, in_=state_psum[:, :, 0:HD + 1])

        # ---------------- own projections ----------------
        q_fm = consts.tile([P, 4, SQ], f32r)
        k_fm = consts.tile([P, 4, SQ], f32r)
        for (w_t, dst) in ((wq_t, q_fm), (wk_t, k_fm)):
            for et in range(4):
                ps = pp.tile([P, SQ], f32, tag="pp")
                chain([nc.tensor.matmul(
                    ps, lhsT=w_t[dt][:, P * et:P * et + P],
                    rhs=xT_own[:, dt, :],
                    start=(dt == 0), stop=(dt == 3)) for dt in range(4)])
                fmap_from_psum(ps, dst[:, et, :], SQ, fmtmp)

        v_own = consts.tile([P, NCH, H, HD + 1], f32r)
        nc.vector.tensor_copy(
            out=v_own[:, :, :, HD:HD + 1],
            in_=onesP_f.rearrange("p (c h o) -> p c h o", c=NCH, h=H))
        for c2 in range(NCH):
            ps = pp.tile([P, D], f32, tag="pp")
            chain([nc.tensor.matmul(
                ps, lhsT=xT_own[:, dt, P * c2:P * c2 + P],
                rhs=wv_t[dt],
                start=(dt == 0), stop=(dt == 3)) for dt in range(4)])
            nc.vector.tensor_copy(
                out=v_own[:, c2, :, 0:HD],
                in_=ps.rearrange("p (h e) -> p h e", h=H))

        # ---------------- attention (one 256-row block, t-subblocks of 128) --
        attn_all = consts.tile([HD, H, SQ], f32r)
        for h in range(H):
            r, p2 = h % 2, h // 2
            qh = q_fm[64 * r:64 * r + 64, p2, :]
            kh = k_fm[64 * r:64 * r + 64, p2, :]

            a01 = pA.tile([P, 2 * SQ], f32, tag="pA")
            chain([
                nc.tensor.matmul(a01[:, 0:SQ], lhsT=kh[:, 0:P],
                                 rhs=qh, start=True, stop=False),
                nc.tensor.matmul(a01[:, SQ:], lhsT=kh[:, P:SQ],
                                 rhs=qh, start=False, stop=True),
            ])
            amc = attnwork.tile([P, 2 * SQ], f32r, tag="amc")
            nc.vector.tensor_mul(amc, a01, maskc)
            am0 = amc[:, 0:SQ]
            am1 = amc[:, SQ:]

            numt = pn.tile([HD + 1, SQ], f32, tag="pn")
            chain([
                nc.tensor.matmul(numt, lhsT=v_own[:, 0, h, :],
                                 rhs=am0, start=True,
                                 stop=False),
                nc.tensor.matmul(numt, lhsT=v_own[:, 1, h, :],
                                 rhs=am1, start=False,
                                 stop=False),
                nc.tensor.matmul(numt,
                                 lhsT=state_sb[64 * r:64 * r + 64, p2, :]
                                 ,
                                 rhs=qh, start=False,
                                 stop=True),
            ])

            rec = attnwork.tile([P, SQ], f32, tag="rec")
            nc.vector.reciprocal(out=rec[64:65, :], in_=numt[HD:HD + 1, :])
            # partition_broadcast only works from partition 0; gpsimd
            # tensor_copy shifts partitions (DVE/ACT cannot)
            rec0 = attnwork.tile([1, SQ], f32, tag="rec0")
            nc.gpsimd.tensor_copy(out=rec0, in_=rec[64:65, :])
            recb = attnwork.tile([HD, SQ], f32, tag="recb")
            nc.gpsimd.partition_broadcast(recb, rec0)
            nc.vector.tensor_mul(attn_all[:, h, :], numt[0:HD, :], recb)

        # ---------------- output projection (fm, for the gate matmul) -------
        outT = consts.tile([P, 4, SQ], f32r)
        for et in range(4):
            ps = pp.tile([P, SQ], f32, tag="pp")
            chain([nc.tensor.matmul(
                ps, lhsT=wo64[:, h, P * et:P * et + P],
                rhs=attn_all[:, h, :],
                start=(h == 0), stop=(h == H - 1)) for h in range(H)])
            nc.vector.tensor_scalar_add(
                out=outT[:, et, :], in0=ps,
                scalar1=bo_fm[:, et:et + 1].bitcast(f32))

        # ---------------- out (row-major) + gate + final mix per chunk ------
        for c2 in range(NCH):
            ps_o = pp.tile([P, D], f32, tag="pp")
            o_mms = [nc.tensor.matmul(
                ps_o, lhsT=attn_all[:, h, P * c2:P * c2 + P],
                rhs=wo64[:, h, :],
                start=(h == 0), stop=False) for h in range(H)]
            o_mms.append(nc.tensor.matmul(ps_o, lhsT=ones1, rhs=bo_row,
                                          start=False, stop=True))
            chain(o_mms)

            # d1 = out - x needs no gate: emitted before the gate matmuls so
            # it overlaps them instead of serializing after the sigmoid
            d1 = outwork.tile([P, D], f32, tag="d1")
            nc.vector.tensor_sub(d1, ps_o, x_rm[:, c2, :])

            # gate in two 256-column halves: the sigmoid + final mix of one
            # half overlaps the other half's matmuls, and the two stores go
            # out on separate HWDGE rings (sync vs scalar)
            y_sb = outwork.tile([P, D], f32, tag="ysb")
            for half in range(2):
                sl = slice(256 * half, 256 * half + 256)
                ps_g = pp.tile([P, SQ], f32, tag="pp")
                g_mms = [nc.tensor.matmul(
                    ps_g, lhsT=xT_own[:, dt, P * c2:P * c2 + P],
                    rhs=wg_t[dt][:, sl],
                    start=(dt == 0), stop=False) for dt in range(4)]
                g_mms += [nc.tensor.matmul(
                    ps_g, lhsT=outT[:, ft, P * c2:P * c2 + P],
                    rhs=wg_t[4 + ft][:, sl],
                    start=False, stop=False) for ft in range(4)]
                g_mms.append(nc.tensor.matmul(
                    ps_g, lhsT=ones1, rhs=bg_row[:, sl],
                    start=False, stop=True))
                chain(g_mms)

                gate_sb = outwork.tile([P, SQ], f32, tag=f"gate{half}")
                nc.scalar.activation(out=gate_sb, in_=ps_g, func=AF.Sigmoid)
                d2 = outwork.tile([P, SQ], f32, tag=f"d2{half}")
                eng = nc.gpsimd if half == 0 else nc.vector
                eng.tensor_mul(d2, gate_sb, d1[:, sl])
                nc.vector.tensor_add(y_sb[:, sl], x_rm[:, c2, sl], d2)
                deng = nc.sync if half == 0 else nc.scalar
                deng.dma_start(
                    out=dy.rearrange("(c p) e -> p c e", p=P)[:, c2, sl],
                    in_=y_sb[:, sl])


def _round_f32r(x):
    # float32r = RNE to 11 mantissa bits (verified against HW cast-DMA)
    xi = x.view(np.uint32).astype(np.uint64)
    bias = ((xi >> 12) & 1) + (1 << 11) - 1
    return ((((xi + bias) >> 12) << 12) & 0xFFFFFFFF).astype(np.uint32).view(np.float32)


def _shard_inputs(inputs):
    x = np.ascontiguousarray(np.asarray(inputs["x"], dtype=np.float32))
    shared = {}
    for name in ("Wq", "Wk", "Wv", "Wo", "bo", "Wg", "bg"):
        shared[name] = _round_f32r(np.ascontiguousarray(
            np.asarray(inputs[name], dtype=np.float32)))
    in_maps = []
    for c in range(NCORE):
        b, j = c // 4, c % 4
        r0 = SQ * j
        x_own = x[b, r0:r0 + SQ]
        x_preT = np.zeros((D, PRE), np.float32)
        x_preT[:, :r0] = x[b, :r0].T
        mask8 = np.zeros((PRE, H), np.float32)
        mask8[:r0] = 1.0
        m = {"x_own": np.ascontiguousarray(x_own),
             "x_ownT": _round_f32r(np.ascontiguousarray(x_own.T)),
             "x_preT": _round_f32r(x_preT), "mask8": mask8}
        m.update(shared)
        in_maps.append(m)
    return in_maps


def kernel(**inputs):
    from concourse import bass_utils

    nc = _build()
    in_maps = _shard_inputs(inputs)
    trace = os.environ.get("BASS_KERNEL_TRACE", "0") == "1"
    res = bass_utils.run_bass_kernel_spmd(
        nc, in_maps, core_ids=list(range(NCORE)), trace=trace)
    LAST_EXEC_NS[0] = res.exec_time_ns
    x = np.asarray(inputs["x"], dtype=np.float32)
    y = np.empty_like(x)
    for c in range(NCORE):
        b, j = c // 4, c % 4
        y[b, SQ * j:SQ * j + SQ] = res.results[c]["y"]
    return y



# revision 38
# speedup vs baseline: 1.5892x; 1.5892x over previous
"""Trainium2 Bass kernel for causal linear attention (elu+1 feature map) with
output projection + sigmoid gate residual mixing.

Reference computation (B=2, S=1024, D=512, H=8, hd=64):
    q = fmap(x@Wq), k = fmap(x@Wk), v = x@Wv          (fmap = elu+1)
    attn[s] = q[s] . cumsum_t<=s(k[t] v[t]^T) / (q[s] . cumsum(k) + 1e-6)
    out = attn@Wo + bo
    gate = sigmoid([x, out]@Wg + bg)
    y = x + gate*(out - x)

Sharding: 8 cores = (b in {0,1}) x (s-quarter j in {0..3}).  Core (b,j) owns
rows [256j, 256j+256) of batch b.  The causal prefix state (sum over earlier
rows of k^T [v|1]) is recomputed locally from a zero-padded prefix input
(uniform SPMD instruction stream; a mask column keeps padding out of the
state).  No cross-core communication.

v2 design notes (all driven by the TimelineSim cost model):
 - everything bf16: halves DMA bytes and avoids the f32r small-N matmul
   penalty; tolerance is 2e-2 so bf16 noise (~3e-3) is fine.
 - heads are processed in PAIRS sharing one PSUM bank: even head at
   partitions 0..63, odd head at 64..127 (via matmul tile_position), so the
   out/gate projections contract K=128 per pair instead of K=64 per head.
 - denominators go to 32-aligned rows of one shared PSUM bank via 1-wide
   ones-matmuls, one batched reciprocal per 4 heads, then a K=1 outer-product
   matmul broadcasts 1/den over 64 partitions (DVE cannot partition-shift).
 - gate uses sigmoid(g) = 0.5*tanh(g/2)+0.5 so ACT needs only table set 0
   (exp/relu/tanh) - no mid-kernel 1283ns table swap.  The 0.5/+-0.5 fold
   into the final mix for free.
 - out@Wg_out is folded into attn@(Wo@Wg_out) (host precomputes the weight
   product), eliminating the transposed out tensor entirely.  bo folds into
   the host-prepared (bo - x) tensor; bg+bo@Wg_out only gets a matmul when
   actually nonzero (it is zeros in this problem).
 - DMA: few large transfers with >=512B contiguous runs (full 360GB/s),
   host supplies every tensor in its exact SBUF layout.
"""

import os
import functools
import numpy as np

B, S, D = 2, 1024, 512
H, HD = 8, 64
SQ = 256          # rows owned per core
PRE = 3 * SQ      # padded prefix rows
NPRE = 6          # prefix chunks of 128
NCH = 2           # own chunks of 128
NCORE = 8
P = 128

LAST_EXEC_NS = [None]


@functools.lru_cache(maxsize=2)
def _build(gate_bias=False):
    import concourse.bass as bass
    import concourse.mybir as mybir
    import concourse.tile as tile
    from concourse import bacc

    f32 = mybir.dt.float32
    bf16 = mybir.dt.bfloat16
    f8 = mybir.dt.float8e4

    nc = bacc.Bacc(
        "TRN2", target_bir_lowering=False, debug=False, num_devices=NCORE
    )

    # consolidated layouts: few big DMAs (HWDGE holds ~630ns each).
    # fp8 tensors carry balanced scales (x/4, W*4) so products are unscaled.
    dxpre = nc.dram_tensor("xpre", [P, NPRE, D], f8, kind="ExternalInput").ap()
    dm8 = nc.dram_tensor("m8", [P, NPRE, H], bf16, kind="ExternalInput").ap()
    # f8w = [xpre chunk0 | wk | wv | wq | xtq8(1024)], all fp8
    df8w = nc.dram_tensor("f8w", [P, 15, D], f8, kind="ExternalInput").ap()
    # (wq/xtq8 live at f8w[9:13] / f8w[13:15])
    dwv16 = nc.dram_tensor("wv16", [P, 4, D], bf16, kind="ExternalInput").ap()
    # xaux = [xtown(1024) | xrm(1024) | c1(1024)]
    dxaux = nc.dram_tensor("xaux", [P, 3, 1024], bf16, kind="ExternalInput").ap()
    # www = [wo | wowg | wgx], each [4, 512]
    dwww = nc.dram_tensor("www", [P, 3, 4, D], bf16, kind="ExternalInput").ap()
    dbg = None
    if gate_bias:
        dbg = nc.dram_tensor("bgrow", [1, D], bf16, kind="ExternalInput").ap()
    dy = nc.dram_tensor("y", [P, NCH, D], bf16, kind="ExternalOutput").ap()

    with tile.TileContext(nc) as tc:
        _emit(nc, tc, mybir, dxpre, dxaux, dm8, df8w, dwv16, dwww, dbg, dy)

    nc.compile()
    return nc


def _emit(nc, tc, mybir, dxpre, dxaux, dm8, df8w, dwv16, dwww, dbg, dy):
    f32 = mybir.dt.float32
    bf16 = mybir.dt.bfloat16
    f8 = mybir.dt.float8e4
    DR = mybir.MatmulPerfMode.DoubleRow
    AF = mybir.ActivationFunctionType
    OP = mybir.AluOpType

    import contextlib
    import bass_rust as _br

    def chain(mms):
        # Accumulating matmuls into one PSUM bank must execute in emission
        # order (start=True first, stop=True last) - the Tile scheduler is
        # otherwise free to reorder same-engine instructions.
        for later, earlier in zip(mms[1:], mms[:-1]):
            _br.add_dep_helper(later.ins, earlier.ins, sync=False,
                               reason="psum accumulation order")

    ctx = contextlib.ExitStack()
    with ctx:
        consts = ctx.enter_context(tc.tile_pool(name="consts", bufs=1))
        fmtmp = ctx.enter_context(tc.tile_pool(name="fmtmp", bufs=4))
        prework = ctx.enter_context(tc.tile_pool(name="prework", bufs=4))
        attnwork = ctx.enter_context(tc.tile_pool(name="attnwork", bufs=3))
        outwork = ctx.enter_context(tc.tile_pool(name="outwork", bufs=4))
        # PSUM pools: concurrent slots must stay <= 8 banks (3+2+2+1)
        pp = ctx.enter_context(tc.tile_pool(name="pp", bufs=3, space="PSUM"))
        pA = ctx.enter_context(tc.tile_pool(name="pA", bufs=2, space="PSUM"))
        pPr = ctx.enter_context(tc.tile_pool(name="pPr", bufs=2, space="PSUM"))
        pDen = ctx.enter_context(tc.tile_pool(name="pDen", bufs=1, space="PSUM"))

        # ---------------- input DMAs (emission order == first-use order) ----
        f8w = consts.tile([P, 15, D], f8)
        nc.sync.dma_start(out=f8w[:, 0:3, :], in_=df8w[:, 0:3, :])
        nc.sync.dma_start(out=f8w[:, 3:5, :], in_=df8w[:, 3:5, :])
        nc.sync.dma_start(out=f8w[:, 5:9, :], in_=df8w[:, 5:9, :])
        xpre_t = consts.tile([P, NPRE, D], f8)
        wk_t = f8w[:, 1:5, :]
        wv_t = f8w[:, 5:9, :]
        wq_t = f8w[:, 9:13, :]
        xtq8 = f8w[:, 13:15, :].rearrange("p a (b s) -> p (a b) s", b=2)
        nc.sync.dma_start(out=xpre_t[:, 1:3, :], in_=dxpre[:, 1:3, :])
        m8 = consts.tile([P, NPRE, H], bf16)
        nc.sync.dma_start(out=m8, in_=dm8)
        nc.sync.dma_start(out=xpre_t[:, 3:6, :], in_=dxpre[:, 3:6, :])
        nc.sync.dma_start(out=f8w[:, 9:13, :], in_=df8w[:, 9:13, :])
        nc.sync.dma_start(out=f8w[:, 13:15, :], in_=df8w[:, 13:15, :])
        wv16_t = consts.tile([P, 4, D], bf16)
        nc.sync.dma_start(out=wv16_t, in_=dwv16)
        xaux = consts.tile([P, 3, 1024], bf16)
        nc.sync.dma_start(out=xaux, in_=dxaux)
        xtown = xaux[:, 0, :].rearrange("p (dt s) -> p dt s", dt=4)
        xrm = xaux[:, 1, :].rearrange("p (c e) -> p c e", c=NCH)
        c1_t = xaux[:, 2, :].rearrange("p (c e) -> p c e", c=NCH)
        www = consts.tile([P, 3, 4, D], bf16)
        nc.sync.dma_start(out=www, in_=dwww)
        wo_t = www[:, 0]
        wowg_t = www[:, 1]
        wgx_t = www[:, 2]
        bg_row = None
        if dbg is not None:
            bg_row = consts.tile([1, D], bf16)
            nc.sync.dma_start(out=bg_row, in_=dbg)

        # ---------------- small device-built constants ----------------
        # combined causal masks: cols 0:256 = [t <= s] for the t0 block
        # (cols 128:256 pass entirely), cols 256:384 = [t <= s-128] for the
        # t1 block restricted to s in [128, 256).
        maskc = consts.tile([P, SQ + P], bf16)
        nc.gpsimd.memset(maskc, 0.0)
        nc.gpsimd.affine_select(
            out=maskc[:, 0:SQ], in_=maskc[:, 0:SQ], compare_op=OP.is_gt,
            fill=1.0, base=0, pattern=[[-1, SQ]], channel_multiplier=1)
        nc.gpsimd.affine_select(
            out=maskc[:, SQ:], in_=maskc[:, SQ:], compare_op=OP.is_gt,
            fill=1.0, base=0, pattern=[[-1, P]], channel_multiplier=1)
        onesb = consts.tile([P, 1], bf16)
        nc.vector.memset(onesb, 1.0)
        ones_full = consts.tile([P, D], bf16)
        nc.vector.memset(ones_full, 1.0)
        ones_col = consts.tile([P, HD], bf16)
        nc.vector.memset(ones_col, 1.0)
        ones1 = None
        if dbg is not None:
            ones1 = consts.tile([1, P], bf16)
            nc.vector.memset(ones1, 1.0)

        # ---------------- prefix state ----------------
        # state[64*(h%2):+64, h//2, :] accumulates K_h^T [V_h | mask] over all
        # prefix chunks (f32 PSUM, copied to bf16 SBUF at the end).
        # full-bank stride (512 f32/partition): partition-offset matmul
        # slices must keep 2048B per-partition pitch for the PSUM
        # pending-zero bookkeeping to line up
        state_psum = pA.tile([P, 4, P], f32, tag="pA")
        state_mms = []
        k_rm_t = {}
        # flat persistent v_pre staging (evacuated via gpsimd cast-DMAs);
        # the state's mask/ksum column comes from a separate m8 matmul
        v_pre_all = consts.tile([P, NPRE, D], bf16)

        def emit_prefix_proj(c):
            xsrc = f8w[:, 0, :] if c == 0 else xpre_t[:, c, :]
            xc = xsrc.rearrange("p (i s) -> p i s", i=4)
            pk_pool = pp if c % 2 == 0 else pPr
            ps_k = pk_pool.tile([P, D], f32,
                                tag="pp" if c % 2 == 0 else "pair",
                                name="ps_k")
            chain([nc.tensor.matmul(
                ps_k, lhsT=xc[:, 2 * j:2 * j + 2, :],
                rhs=wk_t[:, 2 * j:2 * j + 2, :], perf_mode=DR,
                start=(j == 0), stop=(j == 1)) for j in range(2)])
            # fmap = min(exp(t),1) + relu(t), split ACT -> Pool -> DVE
            e_t = fmtmp.tile([P, D], bf16, tag="fm_e")
            nc.scalar.activation(out=e_t, in_=ps_k, func=AF.Exp)
            r_t = fmtmp.tile([P, D], bf16, tag="fm_r")
            nc.scalar.activation(out=r_t, in_=ps_k, func=AF.Relu)
            ec_t = fmtmp.tile([P, D], bf16, tag="fm_ec")
            nc.vector.tensor_tensor(out=ec_t, in0=e_t, in1=ones_full,
                                    op=OP.min)
            k_rm = prework.tile([P, D], bf16, tag="k_rm")
            keng = nc.gpsimd if c in (1, 4) else nc.vector
            keng.tensor_add(k_rm, ec_t, r_t)
            k_rm_t[c] = k_rm

            ps_v = pk_pool.tile([P, D], f32,
                                tag="pp" if c % 2 == 0 else "pair",
                                name="ps_v")
            chain([nc.tensor.matmul(
                ps_v, lhsT=xc[:, 2 * j:2 * j + 2, :],
                rhs=wv_t[:, 2 * j:2 * j + 2, :], perf_mode=DR,
                start=(j == 0), stop=(j == 1)) for j in range(2)])
            if c % 2 == 0:
                nc.scalar.copy(out=v_pre_all[:, c, :], in_=ps_v)
            else:
                nc.vector.tensor_copy(out=v_pre_all[:, c, :], in_=ps_v)

        def emit_state(c):
            k_rm = k_rm_t.pop(c)
            for h in range(H):
                r, p2 = h % 2, h // 2
                # one accumulation group per 64-partition half of the bank
                state_mms.append(nc.tensor.matmul(
                    state_psum[64 * r:64 * r + 64, p2, 0:HD],
                    lhsT=k_rm[:, HD * h:HD * h + HD],
                    rhs=v_pre_all[:, c, HD * h:HD * h + HD],
                    start=(c == 0 and h == r),
                    stop=False,
                    tile_position=(0, 64 * r),
                    skip_group_check=True))
                state_mms.append(nc.tensor.matmul(
                    state_psum[64 * r:64 * r + 64, p2, HD:HD + 1],
                    lhsT=k_rm[:, HD * h:HD * h + HD],
                    rhs=m8[:, c, h:h + 1],
                    start=(c == 0 and h == r),
                    stop=(c == NPRE - 1 and h == H - 2 + r),
                    tile_position=(0, 64 * r),
                    skip_group_check=True))

        emit_prefix_proj(0)
        for c in range(1, NPRE):
            emit_prefix_proj(c)
            if c >= 2:
                emit_state(c - 2)
        # states 4,5 + the state copy are emitted below, interleaved with the
        # first q/k projections so PE never waits on the chunk-5 fmap

        # ---------------- own v projection (row-major) ----------------
        v_own = consts.tile([P, NCH, D], bf16)
        for c2 in range(NCH):
            ps = pp.tile([P, D], f32, tag="pp")
            chain([nc.tensor.matmul(
                ps, lhsT=xtown[:, dt, P * c2:P * c2 + P],
                rhs=wv16_t[:, dt, :],
                start=(dt == 0), stop=(dt == 3)) for dt in range(4)])
            if c2 == 0:
                nc.scalar.copy(out=v_own[:, c2, :], in_=ps)
            else:
                nc.vector.tensor_copy(out=v_own[:, c2, :], in_=ps)

        # ---------------- own q/k (feature-major) + attention --------------
        # q_fm/k_fm [128, 4, 256]: head h lives at partitions 64*(h%2):+64 of
        # feature-tile h//2, which is exactly the paired-head layout.
        q_fm = consts.tile([P, 4, SQ], bf16)
        k_fm = consts.tile([P, 4, SQ], bf16)
        # attn2 [128, 4, 256]: pair p = heads (2p, 2p+1) on partition halves
        attn2 = consts.tile([P, 4, SQ], bf16)
        rec_sb = consts.tile([P, 2, SQ], bf16)
        pair_bank = {}
        den_holder = [None, None]

        def emit_qk(p2, qpool=None):
            # k first: the next a01's Ldweights needs k_fm; engine tables
            # keep the early pairs' critical combines on the fast DVE
            for (w_t, dst) in ((wk_t, k_fm), (wq_t, q_fm)):
                if qpool is not None:
                    ps = qpool.tile([P, SQ], f32, tag="den", name="qk_ps0")
                    qpool = None
                else:
                    ps = pp.tile([P, SQ], f32, tag="pp")
                chain([nc.tensor.matmul(
                    ps, lhsT=w_t[:, 2 * j:2 * j + 2, P * p2:P * p2 + P],
                    rhs=xtq8[:, 2 * j:2 * j + 2, :], perf_mode=DR,
                    start=(j == 0), stop=(j == 1)) for j in range(2)])
                e_t = fmtmp.tile([P, SQ], bf16, tag="fm_e")
                nc.scalar.activation(out=e_t, in_=ps, func=AF.Exp)
                r_t = fmtmp.tile([P, SQ], bf16, tag="fm_r")
                nc.scalar.activation(out=r_t, in_=ps, func=AF.Relu)
                ec_t = fmtmp.tile([P, SQ], bf16, tag="fm_ec")
                nc.vector.tensor_tensor(out=ec_t, in0=e_t,
                                        in1=ones_full[:, 0:SQ], op=OP.min)
                aeng = nc.vector if p2 == 0 else nc.gpsimd
                aeng.tensor_add(dst[:, p2, :], ec_t, r_t)

        def emit_a01(p2):
            """a01 matmuls + causal-mask multiply for both heads of pair p2."""
            amcs = []
            for r in (0, 1):
                h = 2 * p2 + r
                qh = q_fm[64 * r:64 * r + 64, p2, :]
                kh = k_fm[64 * r:64 * r + 64, p2, :]
                a01 = pA.tile([P, 4 * P], f32, tag="pA")
                # two independent single-matmul groups (disjoint regions)
                chain([
                    nc.tensor.matmul(a01[:, 0:SQ], lhsT=kh[:, 0:P], rhs=qh,
                                     start=True, stop=True,
                                     skip_group_check=True),
                    nc.tensor.matmul(a01[:, SQ:SQ + P], lhsT=kh[:, P:SQ],
                                     rhs=qh[:, P:SQ],
                                     start=True, stop=True,
                                     skip_group_check=True),
                ])
                amc = attnwork.tile([P, SQ + P], bf16, tag="amc")
                # the mask-multiply doubles as the PSUM->SBUF move
                nc.vector.tensor_mul(amc, a01[:, 0:SQ + P], maskc)
                amcs.append(amc)
            amc_t[p2] = amcs

        def emit_nd(p2):
            """numerator + denominator matmuls for pair p2."""
            dent, den_memset = den_holder
            pair = pPr.tile([P, 2, SQ], f32, tag="pair")
            pair_bank[p2] = pair
            for r in (0, 1):
                h = 2 * p2 + r
                k4, hf = h % 4, h // 4
                qh = q_fm[64 * r:64 * r + 64, p2, :]
                amc = amc_t[p2][r]
                numt = pair[64 * r:64 * r + 64, 0, :]
                chain([
                    nc.tensor.matmul(numt,
                                     lhsT=v_own[:, 0, HD * h:HD * h + HD],
                                     rhs=amc[:, 0:SQ], start=True, stop=False,
                                     tile_position=(0, 64 * r),
                                     skip_group_check=True),
                    nc.tensor.matmul(pair[64 * r:64 * r + 64, 0, P:SQ],
                                     lhsT=v_own[:, 1, HD * h:HD * h + HD],
                                     rhs=amc[:, SQ:], start=False, stop=False,
                                     tile_position=(0, 64 * r),
                                     skip_group_check=True),
                    nc.tensor.matmul(numt,
                                     lhsT=state_sb[64 * r:64 * r + 64, p2, 0:HD],
                                     rhs=qh, start=False, stop=True,
                                     tile_position=(64 * r, 64 * r),
                                     skip_group_check=True),
                ])
                dh = dent[32 * k4:32 * k4 + 1, hf, :]
                dmm = [
                    nc.tensor.matmul(dh, lhsT=onesb, rhs=amc[:, 0:SQ],
                                     start=True, stop=False,
                                     tile_position=(0, 32 * k4),
                                     skip_group_check=True),
                    nc.tensor.matmul(dent[32 * k4:32 * k4 + 1, hf, P:SQ],
                                     lhsT=onesb, rhs=amc[:, SQ:],
                                     start=False, stop=False,
                                     tile_position=(0, 32 * k4),
                                     skip_group_check=True),
                    nc.tensor.matmul(dh,
                                     lhsT=state_sb[64 * r:64 * r + 64, p2,
                                                   HD:HD + 1],
                                     rhs=qh, start=False, stop=True,
                                     tile_position=(64 * r, 32 * k4),
                                     skip_group_check=True),
                ]
                _br.add_dep_helper(dmm[0].ins, den_memset.ins, sync=False,
                                   reason="den bank preset")
                chain(dmm)

        def emit_recip(hf):
            # batched reciprocal over the 4 dens (rows 0/32/64/96) of half hf
            dent, _ = den_holder
            with nc.allow_low_precision(reason="bf16 1/den, tol 2e-2"):
                nc.vector.reciprocal(out=rec_sb[0:97, hf, :],
                                     in_=dent[0:97, hf, :])

        def emit_finish(p2):
            """1/den broadcast via K=1 matmul + one paired scale->attn2.
            Only one TensorTensor input may be PSUM, so the broadcast is
            copied to SBUF (ACT/DVE alternating) before the scale."""
            pair = pair_bank.pop(p2)
            for r in (0, 1):
                h = 2 * p2 + r
                k4, hf = h % 4, h // 4
                nc.tensor.matmul(
                    pair[64 * r:64 * r + 64, 1, :],
                    lhsT=ones_col[32 * k4:32 * k4 + 1, 0:HD],
                    rhs=rec_sb[32 * k4:32 * k4 + 1, hf, :],
                    start=True, stop=True,
                    tile_position=(32 * k4, 64 * r),
                    skip_group_check=True)
            rec16 = attnwork.tile([P, SQ], bf16, tag="rec16")
            if p2 % 2 == 0:
                nc.scalar.copy(out=rec16, in_=pair[:, 1, :])
            else:
                nc.vector.tensor_copy(out=rec16, in_=pair[:, 1, :])
            nc.vector.tensor_mul(attn2[:, p2, :], pair[:, 0, :], rec16)

        amc_t = {}
        combeng = {0: nc.gpsimd, 1: nc.vector, 2: nc.gpsimd, 3: nc.vector}
        # interleaved so every PE instruction's inputs are ready when the
        # in-order PE sequencer reaches it (fmaps/amc/recip run on
        # ACT/DVE/Pool behind PE).  The first qk psum borrows the den bank
        # (idle until the dens) so two qk pairs are in flight without a 9th
        # PSUM bank.
        emit_qk(0, qpool=pDen)
        emit_state(4)
        den_holder[0] = pDen.tile([P, 2, SQ], f32, tag="den", name="dent_t")
        den_holder[1] = nc.vector.memset(den_holder[0], 1.0)
        emit_qk(1)
        emit_state(5)
        chain(state_mms)
        state_sb = consts.tile([P, 4, HD + 1], bf16)
        nc.scalar.copy(out=state_sb, in_=state_psum[:, :, 0:HD + 1])
        emit_a01(0)
        emit_qk(2)
        emit_nd(0)
        emit_a01(1)
        emit_qk(3)
        emit_nd(1)
        emit_recip(0)
        emit_a01(2)
        emit_finish(0)
        emit_a01(3)
        emit_nd(2)
        emit_finish(1)
        emit_nd(3)
        emit_recip(1)

        # out projection: per-pair matmuls double as gap fillers around the
        # second reciprocal / final bcasts
        o_mms = {0: [], 1: []}
        o_ps = {}

        def emit_out_mm(c2, pr):
            if c2 not in o_ps:
                ps_o_ = pp.tile([P, D], f32, tag="pp", name=f"ps_o{c2}")
                o_ps[c2] = ps_o_
            o_mms[c2].append(nc.tensor.matmul(
                o_ps[c2], lhsT=attn2[:, pr, P * c2:P * c2 + P],
                rhs=wo_t[:, pr, :],
                start=(pr == 0), stop=(pr == 3)))

        emit_out_mm(0, 0)
        emit_out_mm(0, 1)
        emit_finish(2)
        emit_out_mm(0, 2)
        emit_finish(3)
        emit_out_mm(0, 3)
        for pr in range(4):
            emit_out_mm(1, pr)
        chain(o_mms[0])
        chain(o_mms[1])

        # ---------------- gate + final mix ----------------
        y_sb = outwork.tile([P, NCH, D], bf16, tag="ysb")
        d1_t = {}
        for c2 in range(NCH):
            # d1 = 0.5*(out - x + bo)  (c1 = 0.5*(bo - x) precomputed on host)
            d1 = outwork.tile([P, D], bf16, tag=f"d1_{c2}")
            nc.vector.scalar_tensor_tensor(
                out=d1, in0=o_ps[c2], scalar=0.5, in1=c1_t[:, c2, :],
                op0=OP.mult, op1=OP.add)
            u_t = outwork.tile([P, D], bf16, tag=f"u_{c2}")
            nc.vector.tensor_add(u_t, d1, xrm[:, c2, :])
            d1_t[c2] = (d1, u_t)

        for c2 in range(NCH):
            d1, u_t = d1_t[c2]
            for half in range(2):
                sl = slice(SQ * half, SQ * half + SQ)
                ps_g = pA.tile([P, SQ], f32, tag="pA")
                g_mms = [nc.tensor.matmul(
                    ps_g, lhsT=xtown[:, dt, P * c2:P * c2 + P],
                    rhs=wgx_t[:, dt, sl],
                    start=(dt == 0), stop=False) for dt in range(4)]
                g_mms += [nc.tensor.matmul(
                    ps_g, lhsT=attn2[:, pr, P * c2:P * c2 + P],
                    rhs=wowg_t[:, pr, sl],
                    start=False, stop=(dbg is None and pr == 3))
                    for pr in range(4)]
                if dbg is not None:
                    g_mms.append(nc.tensor.matmul(
                        ps_g, lhsT=ones1, rhs=bg_row[:, sl],
                        start=False, stop=True))
                chain(g_mms)

                # gate = 0.5*tanh(g/2) + 0.5;  y = x + gate*(out-x)
                #      = x + (tanh(g/2)+1)*d1  with d1 pre-halved
                # tail-latency tuning: the penultimate half's mix goes to
                # Pool so DVE is free for the final half, and the final half
                # is split into two 128-col chunks so the last store waits on
                # as little as possible
                # y = x + (t+1)*d1 = (x + d1) + t*d1 = u + t*d1
                zeng = nc.vector
                t_sb = outwork.tile([P, SQ], bf16, tag=f"t{half}")
                nc.scalar.activation(out=t_sb, in_=ps_g, func=AF.Tanh,
                                     scale=0.5)
                z_sb = outwork.tile([P, SQ], bf16, tag=f"z{half}")
                zeng.tensor_mul(z_sb, t_sb, d1[:, sl])
                nc.vector.tensor_add(y_sb[:, c2, sl], z_sb, u_t[:, sl])
                if c2 == 0 and half == 0:
                    pass  # stored together with half 1 (one HWDGE hold)
                elif c2 == 0:
                    nc.sync.dma_start(out=dy[:, 0, :], in_=y_sb[:, 0, :])
                else:
                    nc.sync.dma_start(out=dy[:, c2, sl], in_=y_sb[:, c2, sl])


def _to_bf16(x):
    import ml_dtypes
    return np.asarray(x, dtype=np.float32).astype(ml_dtypes.bfloat16)


def _to_f8(x):
    import ml_dtypes
    return np.asarray(x, dtype=np.float32).astype(ml_dtypes.float8_e4m3)


def _shard_inputs(inputs, gate_bias):
    x = np.ascontiguousarray(np.asarray(inputs["x"], dtype=np.float32))
    Wq = np.asarray(inputs["Wq"], dtype=np.float32)
    Wk = np.asarray(inputs["Wk"], dtype=np.float32)
    Wv = np.asarray(inputs["Wv"], dtype=np.float32)
    Wo = np.asarray(inputs["Wo"], dtype=np.float32)
    bo = np.asarray(inputs["bo"], dtype=np.float32)
    Wg = np.asarray(inputs["Wg"], dtype=np.float32)
    bg = np.asarray(inputs["bg"], dtype=np.float32)

    def wtile(w):
        # [512, 512] -> [128 p, 4 dt, 512 e] with dt = row//128
        return np.ascontiguousarray(
            _to_bf16(w).reshape(4, P, D).transpose(1, 0, 2))

    def wtile8(w):
        # fp8 with balanced x4 scale (x side carries /4)
        return np.ascontiguousarray(
            _to_f8(4.0 * w).reshape(4, P, D).transpose(1, 0, 2))

    WoWg = Wo @ Wg[D:]
    bg_eff = bg + bo @ Wg[D:]
    www = np.stack([wtile(Wo), wtile(WoWg), wtile(Wg[:D])], axis=1)
    wkvq8 = np.concatenate([wtile8(Wk), wtile8(Wv), wtile8(Wq)],
                           axis=1)  # [P, 12, D]
    shared = {
        "wkvq8": wkvq8,
        "wv16": wtile(Wv),
        "www": np.ascontiguousarray(www),
    }
    if gate_bias:
        shared["bgrow"] = _to_bf16(bg_eff).reshape(1, D)

    in_maps = []
    for cidx in range(NCORE):
        b, j = cidx // 4, cidx % 4
        r0 = SQ * j
        xb = x[b]
        # prefix, zero-padded to 768 rows, chunk-major feature-major layout:
        # xpre[p, c, dt*128+s] = x[b, 128c+s, 128dt+p]
        xpre = np.zeros((PRE, D), np.float32)
        xpre[:r0] = xb[:r0]
        xpre_t = xpre.reshape(NPRE, P, 4, P).transpose(3, 0, 2, 1).reshape(
            P, NPRE, D)
        xo = xb[r0:r0 + SQ]
        # xtown[p, dt, s] = x[b, r0+s, 128dt+p]
        xtown = xo.reshape(SQ, 4, P).transpose(2, 1, 0)
        # row-major own rows: [p, c2, e] = x[b, r0+128c2+p, e]
        xrm = xo.reshape(NCH, P, D).transpose(1, 0, 2)
        c1 = 0.5 * (bo[None, None, :] - xrm)
        m8 = np.zeros((PRE, H), np.float32)
        m8[:r0] = 1.0
        m8_t = m8.reshape(NPRE, P, H).transpose(1, 0, 2)
        xaux = np.stack([xtown.reshape(P, 1024), xrm.reshape(P, 1024),
                         c1.reshape(P, 1024)], axis=1)
        xpre8 = _to_f8(xpre_t / 4.0)
        xtq8 = _to_f8(xtown / 4.0).reshape(P, 2, D)
        f8w = np.concatenate([xpre8[:, 0:1, :], shared["wkvq8"], xtq8],
                             axis=1)
        m = {"xpre": np.ascontiguousarray(xpre8),
             "f8w": np.ascontiguousarray(f8w),
             "xaux": np.ascontiguousarray(_to_bf16(xaux)),
             "m8": np.ascontiguousarray(_to_bf16(m8_t))}
        m.update({k: v for k, v in shared.items() if k != "wkvq8"})
        in_maps.append(m)
    return in_maps


def kernel(**inputs):
    from concourse import bass_utils

    bo = np.asarray(inputs["bo"], dtype=np.float32)
    bg = np.asarray(inputs["bg"], dtype=np.float32)
    Wg = np.asarray(inputs["Wg"], dtype=np.float32)
    gate_bias = bool(np.any(bg + bo @ Wg[D:]))

    nc = _build(gate_bias)
    in_maps = _shard_inputs(inputs, gate_bias)
    trace = os.environ.get("BASS_KERNEL_TRACE", "0") == "1"
    res = bass_utils.run_bass_kernel_spmd(
        nc, in_maps, core_ids=list(range(NCORE)), trace=trace)
    LAST_EXEC_NS[0] = res.exec_time_ns
    x = np.asarray(inputs["x"], dtype=np.float32)
    y = np.empty_like(x)
    for cidx in range(NCORE):
        b, j = cidx // 4, cidx % 4
        yc = np.asarray(res.results[cidx]["y"], dtype=np.float32)
        y[b, SQ * j:SQ * j + SQ] = yc.transpose(1, 0, 2).reshape(SQ, D)
    return y


# revision 41
# speedup vs baseline: 1.5914x; 1.0014x over previous
"""Trainium2 Bass kernel for causal linear attention (elu+1 feature map) with
output projection + sigmoid gate residual mixing.

Reference computation (B=2, S=1024, D=512, H=8, hd=64):
    q = fmap(x@Wq), k = fmap(x@Wk), v = x@Wv          (fmap = elu+1)
    attn[s] = q[s] . cumsum_t<=s(k[t] v[t]^T) / (q[s] . cumsum(k) + 1e-6)
    out = attn@Wo + bo
    gate = sigmoid([x, out]@Wg + bg)
    y = x + gate*(out - x)

Sharding: 8 cores = (b in {0,1}) x (s-quarter j in {0..3}).  Core (b,j) owns
rows [256j, 256j+256) of batch b.  The causal prefix state (sum over earlier
rows of k^T [v|1]) is recomputed locally from a zero-padded prefix input
(uniform SPMD instruction stream; a mask column keeps padding out of the
state).  No cross-core communication.

Design notes (driven by the TimelineSim cost model, which is what the
harness reports as HW exec time):
 - fp8 e4m3 DoubleRow matmuls (0.5 cycles/row, two K-tiles per matmul = 4x
   over bf16) for the prefix k/v and own q/k projections; balanced scaling
   (x/4, W*4) keeps products unscaled and avoids e4m3 subnormals.  The q/k
   fp8 noise cancels in the attention num/den ratio; v/out/gate paths stay
   bf16 (fp8 there breaks the 2e-2 gate).  Verified rel_err 1.6e-2 on the
   PJRT path (worst core j=1: short prefix averages fp8 state noise least).
 - everything else bf16: halves DMA bytes and avoids the f32r small-N
   matmul penalty.
 - heads are processed in PAIRS sharing one PSUM bank: even head at
   partitions 0..63, odd head at 64..127 (via matmul tile_position), so the
   out/gate projections contract K=128 per pair instead of K=64 per head.
 - denominators go to 32-aligned rows of one shared PSUM bank via 1-wide
   ones-matmuls, one batched reciprocal per 4 heads, then a K=1 outer-product
   matmul broadcasts 1/den over 64 partitions (DVE cannot partition-shift).
 - gate uses sigmoid(g) = 0.5*tanh(g/2)+0.5 so ACT needs only table set 0
   (exp/relu/tanh) - no mid-kernel 1283ns table swap; the 0.5 folds into the
   host-prepared c1 = 0.5*(bo-x) and the final mix y = (x+d1) + tanh*d1.
 - out@Wg_out is folded into attn@(Wo@Wg_out) (host precomputes the weight
   product), eliminating the transposed out tensor entirely.  bg+bo@Wg_out
   only gets a matmul when actually nonzero (it is zeros in this problem).
 - DMA: few large transfers with >=512B contiguous runs (full 360GB/s),
   host supplies every tensor in its exact SBUF layout (fp8 tensors share
   one blob so the first transfer covers xpre chunk0 + Wk's first half).
 - engine legality on real HW (BIR verifier): Pool/GPSIMD cannot touch PSUM
   and only supports TensorTensor add/mult + copy/memset/affine_select on
   SBUF; TensorTensor may read at most ONE PSUM operand; matmul tile row
   position must match the stationary operand's start partition.  All PSUM
   evacuation therefore lives on ACT/DVE, with the fmap split
   exp/relu(ACT) -> min(DVE 2x TT) -> add(Pool TT) to use all engines.
"""

import os
import functools
import numpy as np

B, S, D = 2, 1024, 512
H, HD = 8, 64
SQ = 256          # rows owned per core
PRE = 3 * SQ      # padded prefix rows
NPRE = 6          # prefix chunks of 128
NCH = 2           # own chunks of 128
NCORE = 8
P = 128

LAST_EXEC_NS = [None]


@functools.lru_cache(maxsize=2)
def _build(gate_bias=False):
    import concourse.bass as bass
    import concourse.mybir as mybir
    import concourse.tile as tile
    from concourse import bacc

    f32 = mybir.dt.float32
    bf16 = mybir.dt.bfloat16
    f8 = mybir.dt.float8e4

    nc = bacc.Bacc(
        "TRN2", target_bir_lowering=False, debug=False, num_devices=NCORE
    )

    # consolidated layouts: few big DMAs (HWDGE holds ~630ns each).
    # fp8 tensors carry balanced scales (x/4, W*4) so products are unscaled.
    dxpre = nc.dram_tensor("xpre", [P, NPRE, D], f8, kind="ExternalInput").ap()
    dm8 = nc.dram_tensor("m8", [P, NPRE, H], bf16, kind="ExternalInput").ap()
    # f8w = [xpre chunk0 | wk | wv | wq | xtq8(1024)], all fp8
    df8w = nc.dram_tensor("f8w", [P, 15, D], f8, kind="ExternalInput").ap()
    # (wq/xtq8 live at f8w[9:13] / f8w[13:15])
    dwv16 = nc.dram_tensor("wv16", [P, 4, D], bf16, kind="ExternalInput").ap()
    # xaux = [xtown(1024) | xrm(1024) | c1(1024)]
    dxaux = nc.dram_tensor("xaux", [P, 3, 1024], bf16, kind="ExternalInput").ap()
    # www = [wo | wowg | wgx], each [4, 512]
    dwww = nc.dram_tensor("www", [P, 3, 4, D], bf16, kind="ExternalInput").ap()
    dbg = None
    if gate_bias:
        dbg = nc.dram_tensor("bgrow", [1, D], bf16, kind="ExternalInput").ap()
    dy = nc.dram_tensor("y", [P, NCH, D], bf16, kind="ExternalOutput").ap()

    with tile.TileContext(nc) as tc:
        _emit(nc, tc, mybir, dxpre, dxaux, dm8, df8w, dwv16, dwww, dbg, dy)

    nc.compile()
    return nc


def _emit(nc, tc, mybir, dxpre, dxaux, dm8, df8w, dwv16, dwww, dbg, dy):
    f32 = mybir.dt.float32
    bf16 = mybir.dt.bfloat16
    f8 = mybir.dt.float8e4
    DR = mybir.MatmulPerfMode.DoubleRow
    AF = mybir.ActivationFunctionType
    OP = mybir.AluOpType

    import contextlib
    import bass_rust as _br

    def chain(mms):
        # Accumulating matmuls into one PSUM bank must execute in emission
        # order (start=True first, stop=True last) - the Tile scheduler is
        # otherwise free to reorder same-engine instructions.
        for later, earlier in zip(mms[1:], mms[:-1]):
            _br.add_dep_helper(later.ins, earlier.ins, sync=False,
                               reason="psum accumulation order")

    ctx = contextlib.ExitStack()
    with ctx:
        consts = ctx.enter_context(tc.tile_pool(name="consts", bufs=1))
        fmtmp = ctx.enter_context(tc.tile_pool(name="fmtmp", bufs=4))
        prework = ctx.enter_context(tc.tile_pool(name="prework", bufs=4))
        attnwork = ctx.enter_context(tc.tile_pool(name="attnwork", bufs=3))
        outwork = ctx.enter_context(tc.tile_pool(name="outwork", bufs=4))
        # PSUM pools: concurrent slots must stay <= 8 banks (3+2+2+1)
        pp = ctx.enter_context(tc.tile_pool(name="pp", bufs=3, space="PSUM"))
        pA = ctx.enter_context(tc.tile_pool(name="pA", bufs=2, space="PSUM"))
        pPr = ctx.enter_context(tc.tile_pool(name="pPr", bufs=2, space="PSUM"))
        pDen = ctx.enter_context(tc.tile_pool(name="pDen", bufs=1, space="PSUM"))

        # ---------------- input DMAs (emission order == first-use order) ----
        f8w = consts.tile([P, 15, D], f8)
        nc.sync.dma_start(out=f8w[:, 0:3, :], in_=df8w[:, 0:3, :])
        nc.sync.dma_start(out=f8w[:, 3:5, :], in_=df8w[:, 3:5, :])
        nc.sync.dma_start(out=f8w[:, 5:9, :], in_=df8w[:, 5:9, :])
        xpre_t = consts.tile([P, NPRE, D], f8)
        wk_t = f8w[:, 1:5, :]
        wv_t = f8w[:, 5:9, :]
        wq_t = f8w[:, 9:13, :]
        xtq8 = f8w[:, 13:15, :].rearrange("p a (b s) -> p (a b) s", b=2)
        nc.sync.dma_start(out=xpre_t[:, 1:3, :], in_=dxpre[:, 1:3, :])
        m8 = consts.tile([P, NPRE, H], bf16)
        nc.sync.dma_start(out=m8, in_=dm8)
        nc.sync.dma_start(out=xpre_t[:, 3:6, :], in_=dxpre[:, 3:6, :])
        nc.sync.dma_start(out=f8w[:, 9:13, :], in_=df8w[:, 9:13, :])
        nc.sync.dma_start(out=f8w[:, 13:15, :], in_=df8w[:, 13:15, :])
        xaux = consts.tile([P, 3, 1024], bf16)
        nc.sync.dma_start(out=xaux[:, 0, :], in_=dxaux[:, 0, :])
        wv16_t = consts.tile([P, 4, D], bf16)
        nc.sync.dma_start(out=wv16_t, in_=dwv16)
        nc.sync.dma_start(out=xaux[:, 1:3, :], in_=dxaux[:, 1:3, :])
        xtown = xaux[:, 0, :].rearrange("p (dt s) -> p dt s", dt=4)
        xrm = xaux[:, 1, :].rearrange("p (c e) -> p c e", c=NCH)
        c1_t = xaux[:, 2, :].rearrange("p (c e) -> p c e", c=NCH)
        www = consts.tile([P, 3, 4, D], bf16)
        nc.sync.dma_start(out=www, in_=dwww)
        wo_t = www[:, 0]
        wowg_t = www[:, 1]
        wgx_t = www[:, 2]
        bg_row = None
        if dbg is not None:
            bg_row = consts.tile([1, D], bf16)
            nc.sync.dma_start(out=bg_row, in_=dbg)

        # ---------------- small device-built constants ----------------
        # combined causal masks: cols 0:256 = [t <= s] for the t0 block
        # (cols 128:256 pass entirely), cols 256:384 = [t <= s-128] for the
        # t1 block restricted to s in [128, 256).
        maskc = consts.tile([P, SQ + P], bf16)
        nc.gpsimd.memset(maskc, 0.0)
        nc.gpsimd.affine_select(
            out=maskc[:, 0:SQ], in_=maskc[:, 0:SQ], compare_op=OP.is_gt,
            fill=1.0, base=0, pattern=[[-1, SQ]], channel_multiplier=1)
        nc.gpsimd.affine_select(
            out=maskc[:, SQ:], in_=maskc[:, SQ:], compare_op=OP.is_gt,
            fill=1.0, base=0, pattern=[[-1, P]], channel_multiplier=1)
        onesb = consts.tile([P, 1], bf16)
        nc.vector.memset(onesb, 1.0)
        ones_full = consts.tile([P, D], bf16)
        nc.vector.memset(ones_full, 1.0)
        ones_col = consts.tile([P, HD], bf16)
        nc.vector.memset(ones_col, 1.0)
        ones1 = None
        if dbg is not None:
            ones1 = consts.tile([1, P], bf16)
            nc.vector.memset(ones1, 1.0)

        # ---------------- prefix state ----------------
        # state[64*(h%2):+64, h//2, :] accumulates K_h^T [V_h | mask] over all
        # prefix chunks (f32 PSUM, copied to bf16 SBUF at the end).
        # full-bank stride (512 f32/partition): partition-offset matmul
        # slices must keep 2048B per-partition pitch for the PSUM
        # pending-zero bookkeeping to line up
        state_psum = pA.tile([P, 4, P], f32, tag="pA")
        state_mms = []
        k_rm_t = {}
        # flat persistent v_pre staging (evacuated via gpsimd cast-DMAs);
        # the state's mask/ksum column comes from a separate m8 matmul
        v_pre_all = consts.tile([P, NPRE, D], bf16)

        def emit_prefix_proj(c):
            xsrc = f8w[:, 0, :] if c == 0 else xpre_t[:, c, :]
            xc = xsrc.rearrange("p (i s) -> p i s", i=4)
            pk_pool = pp if c % 2 == 0 else pPr
            ps_k = pk_pool.tile([P, D], f32,
                                tag="pp" if c % 2 == 0 else "pair",
                                name="ps_k")
            chain([nc.tensor.matmul(
                ps_k, lhsT=xc[:, 2 * j:2 * j + 2, :],
                rhs=wk_t[:, 2 * j:2 * j + 2, :], perf_mode=DR,
                start=(j == 0), stop=(j == 1)) for j in range(2)])
            # fmap = min(exp(t),1) + relu(t), split ACT -> Pool -> DVE
            e_t = fmtmp.tile([P, D], bf16, tag="fm_e")
            nc.scalar.activation(out=e_t, in_=ps_k, func=AF.Exp)
            r_t = fmtmp.tile([P, D], bf16, tag="fm_r")
            nc.scalar.activation(out=r_t, in_=ps_k, func=AF.Relu)
            ec_t = fmtmp.tile([P, D], bf16, tag="fm_ec")
            nc.vector.tensor_tensor(out=ec_t, in0=e_t, in1=ones_full,
                                    op=OP.min)
            k_rm = prework.tile([P, D], bf16, tag="k_rm")
            keng = nc.gpsimd if c in (3, 4) else nc.vector
            keng.tensor_add(k_rm, ec_t, r_t)
            k_rm_t[c] = k_rm

            ps_v = pk_pool.tile([P, D], f32,
                                tag="pp" if c % 2 == 0 else "pair",
                                name="ps_v")
            chain([nc.tensor.matmul(
                ps_v, lhsT=xc[:, 2 * j:2 * j + 2, :],
                rhs=wv_t[:, 2 * j:2 * j + 2, :], perf_mode=DR,
                start=(j == 0), stop=(j == 1)) for j in range(2)])
            if c % 2 == 0:
                nc.scalar.copy(out=v_pre_all[:, c, :], in_=ps_v)
            else:
                nc.vector.tensor_copy(out=v_pre_all[:, c, :], in_=ps_v)

        def emit_state(c):
            k_rm = k_rm_t.pop(c)
            for h in range(H):
                r, p2 = h % 2, h // 2
                # one accumulation group per 64-partition half of the bank
                state_mms.append(nc.tensor.matmul(
                    state_psum[64 * r:64 * r + 64, p2, 0:HD],
                    lhsT=k_rm[:, HD * h:HD * h + HD],
                    rhs=v_pre_all[:, c, HD * h:HD * h + HD],
                    start=(c == 0 and h == r),
                    stop=False,
                    tile_position=(0, 64 * r),
                    skip_group_check=True))
                state_mms.append(nc.tensor.matmul(
                    state_psum[64 * r:64 * r + 64, p2, HD:HD + 1],
                    lhsT=k_rm[:, HD * h:HD * h + HD],
                    rhs=m8[:, c, h:h + 1],
                    start=(c == 0 and h == r),
                    stop=(c == NPRE - 1 and h == H - 2 + r),
                    tile_position=(0, 64 * r),
                    skip_group_check=True))

        emit_prefix_proj(0)
        for c in range(1, NPRE):
            emit_prefix_proj(c)
            if c >= 2:
                emit_state(c - 2)
        # states 4,5 + the state copy are emitted below, interleaved with the
        # first q/k projections so PE never waits on the chunk-5 fmap

        # ---------------- own v projection (row-major) ----------------
        v_own = consts.tile([P, NCH, D], bf16)
        for c2 in range(NCH):
            ps = pp.tile([P, D], f32, tag="pp")
            chain([nc.tensor.matmul(
                ps, lhsT=xtown[:, dt, P * c2:P * c2 + P],
                rhs=wv16_t[:, dt, :],
                start=(dt == 0), stop=(dt == 3)) for dt in range(4)])
            if c2 == 0:
                nc.scalar.copy(out=v_own[:, c2, :], in_=ps)
            else:
                nc.vector.tensor_copy(out=v_own[:, c2, :], in_=ps)

        # ---------------- own q/k (feature-major) + attention --------------
        # q_fm/k_fm [128, 4, 256]: head h lives at partitions 64*(h%2):+64 of
        # feature-tile h//2, which is exactly the paired-head layout.
        q_fm = consts.tile([P, 4, SQ], bf16)
        k_fm = consts.tile([P, 4, SQ], bf16)
        # attn2 [128, 4, 256]: pair p = heads (2p, 2p+1) on partition halves
        attn2 = consts.tile([P, 4, SQ], bf16)
        rec_sb = consts.tile([P, 2, SQ], bf16)
        pair_bank = {}
        den_holder = [None, None]

        def emit_qk(p2, qpool=None):
            # k first: the next a01's Ldweights needs k_fm; engine tables
            # keep the early pairs' critical combines on the fast DVE
            for (w_t, dst) in ((wk_t, k_fm), (wq_t, q_fm)):
                if qpool is not None:
                    ps = qpool.tile([P, SQ], f32, tag="den", name="qk_ps0")
                    qpool = None
                else:
                    ps = pp.tile([P, SQ], f32, tag="pp")
                chain([nc.tensor.matmul(
                    ps, lhsT=w_t[:, 2 * j:2 * j + 2, P * p2:P * p2 + P],
                    rhs=xtq8[:, 2 * j:2 * j + 2, :], perf_mode=DR,
                    start=(j == 0), stop=(j == 1)) for j in range(2)])
                e_t = fmtmp.tile([P, SQ], bf16, tag="fm_e")
                nc.scalar.activation(out=e_t, in_=ps, func=AF.Exp)
                r_t = fmtmp.tile([P, SQ], bf16, tag="fm_r")
                nc.scalar.activation(out=r_t, in_=ps, func=AF.Relu)
                ec_t = fmtmp.tile([P, SQ], bf16, tag="fm_ec")
                nc.vector.tensor_tensor(out=ec_t, in0=e_t,
                                        in1=ones_full[:, 0:SQ], op=OP.min)
                aeng = nc.vector if p2 == 0 else nc.gpsimd
                aeng.tensor_add(dst[:, p2, :], ec_t, r_t)

        def emit_a01(p2):
            """a01 matmuls + causal-mask multiply for both heads of pair p2."""
            amcs = []
            for r in (0, 1):
                h = 2 * p2 + r
                qh = q_fm[64 * r:64 * r + 64, p2, :]
                kh = k_fm[64 * r:64 * r + 64, p2, :]
                a01 = pA.tile([P, 4 * P], f32, tag="pA")
                # two independent single-matmul groups (disjoint regions)
                chain([
                    nc.tensor.matmul(a01[:, 0:SQ], lhsT=kh[:, 0:P], rhs=qh,
                                     start=True, stop=True,
                                     skip_group_check=True),
                    nc.tensor.matmul(a01[:, SQ:SQ + P], lhsT=kh[:, P:SQ],
                                     rhs=qh[:, P:SQ],
                                     start=True, stop=True,
                                     skip_group_check=True),
                ])
                amc = attnwork.tile([P, SQ + P], bf16, tag="amc")
                # the mask-multiply doubles as the PSUM->SBUF move
                nc.vector.tensor_mul(amc, a01[:, 0:SQ + P], maskc)
                amcs.append(amc)
            amc_t[p2] = amcs

        def emit_nd(p2):
            """numerator + denominator matmuls for pair p2."""
            dent, den_memset = den_holder
            pair = pPr.tile([P, 2, SQ], f32, tag="pair")
            pair_bank[p2] = pair
            for r in (0, 1):
                h = 2 * p2 + r
                k4, hf = h % 4, h // 4
                qh = q_fm[64 * r:64 * r + 64, p2, :]
                amc = amc_t[p2][r]
                numt = pair[64 * r:64 * r + 64, 0, :]
                chain([
                    nc.tensor.matmul(numt,
                                     lhsT=v_own[:, 0, HD * h:HD * h + HD],
                                     rhs=amc[:, 0:SQ], start=True, stop=False,
                                     tile_position=(0, 64 * r),
                                     skip_group_check=True),
                    nc.tensor.matmul(pair[64 * r:64 * r + 64, 0, P:SQ],
                                     lhsT=v_own[:, 1, HD * h:HD * h + HD],
                                     rhs=amc[:, SQ:], start=False, stop=False,
                                     tile_position=(0, 64 * r),
                                     skip_group_check=True),
                    nc.tensor.matmul(numt,
                                     lhsT=state_sb[64 * r:64 * r + 64, p2, 0:HD],
                                     rhs=qh, start=False, stop=True,
                                     tile_position=(64 * r, 64 * r),
                                     skip_group_check=True),
                ])
                dh = dent[32 * k4:32 * k4 + 1, hf, :]
                dmm = [
                    nc.tensor.matmul(dh, lhsT=onesb, rhs=amc[:, 0:SQ],
                                     start=True, stop=False,
                                     tile_position=(0, 32 * k4),
                                     skip_group_check=True),
                    nc.tensor.matmul(dent[32 * k4:32 * k4 + 1, hf, P:SQ],
                                     lhsT=onesb, rhs=amc[:, SQ:],
                                     start=False, stop=False,
                                     tile_position=(0, 32 * k4),
                                     skip_group_check=True),
                    nc.tensor.matmul(dh,
                                     lhsT=state_sb[64 * r:64 * r + 64, p2,
                                                   HD:HD + 1],
                                     rhs=qh, start=False, stop=True,
                                     tile_position=(64 * r, 32 * k4),
                                     skip_group_check=True),
                ]
                _br.add_dep_helper(dmm[0].ins, den_memset.ins, sync=False,
                                   reason="den bank preset")
                chain(dmm)

        def emit_recip(hf):
            # batched reciprocal over the 4 dens (rows 0/32/64/96) of half hf
            dent, _ = den_holder
            with nc.allow_low_precision(reason="bf16 1/den, tol 2e-2"):
                nc.vector.reciprocal(out=rec_sb[0:97, hf, :],
                                     in_=dent[0:97, hf, :])

        def emit_finish(p2):
            """1/den broadcast via K=1 matmul + one paired scale->attn2.
            Only one TensorTensor input may be PSUM, so the broadcast is
            copied to SBUF (ACT/DVE alternating) before the scale."""
            pair = pair_bank.pop(p2)
            for r in (0, 1):
                h = 2 * p2 + r
                k4, hf = h % 4, h // 4
                nc.tensor.matmul(
                    pair[64 * r:64 * r + 64, 1, :],
                    lhsT=ones_col[32 * k4:32 * k4 + 1, 0:HD],
                    rhs=rec_sb[32 * k4:32 * k4 + 1, hf, :],
                    start=True, stop=True,
                    tile_position=(32 * k4, 64 * r),
                    skip_group_check=True)
            rec16 = attnwork.tile([P, SQ], bf16, tag="rec16")
            if p2 % 2 == 0:
                nc.scalar.copy(out=rec16, in_=pair[:, 1, :])
            else:
                nc.vector.tensor_copy(out=rec16, in_=pair[:, 1, :])
            nc.vector.tensor_mul(attn2[:, p2, :], pair[:, 0, :], rec16)

        amc_t = {}
        combeng = {0: nc.gpsimd, 1: nc.vector, 2: nc.gpsimd, 3: nc.vector}
        # interleaved so every PE instruction's inputs are ready when the
        # in-order PE sequencer reaches it (fmaps/amc/recip run on
        # ACT/DVE/Pool behind PE).  The first qk psum borrows the den bank
        # (idle until the dens) so two qk pairs are in flight without a 9th
        # PSUM bank.
        emit_qk(0, qpool=pDen)
        emit_state(4)
        den_holder[0] = pDen.tile([P, 2, SQ], f32, tag="den", name="dent_t")
        den_holder[1] = nc.vector.memset(den_holder[0], 1.0)
        emit_qk(1)
        emit_state(5)
        chain(state_mms)
        state_sb = consts.tile([P, 4, HD + 1], bf16)
        nc.scalar.copy(out=state_sb, in_=state_psum[:, :, 0:HD + 1])
        emit_a01(0)
        emit_qk(2)
        emit_nd(0)
        emit_a01(1)
        emit_qk(3)
        emit_nd(1)
        emit_recip(0)
        emit_a01(2)
        emit_finish(0)
        emit_a01(3)
        emit_nd(2)
        emit_finish(1)
        emit_nd(3)
        emit_recip(1)

        # out projection: per-pair matmuls double as gap fillers around the
        # second reciprocal / final bcasts
        o_mms = {0: [], 1: []}
        o_ps = {}

        def emit_out_mm(c2, pr):
            if c2 not in o_ps:
                ps_o_ = pp.tile([P, D], f32, tag="pp", name=f"ps_o{c2}")
                o_ps[c2] = ps_o_
            o_mms[c2].append(nc.tensor.matmul(
                o_ps[c2], lhsT=attn2[:, pr, P * c2:P * c2 + P],
                rhs=wo_t[:, pr, :],
                start=(pr == 0), stop=(pr == 3)))

        emit_out_mm(0, 0)
        emit_out_mm(0, 1)
        emit_finish(2)
        emit_out_mm(0, 2)
        emit_finish(3)
        emit_out_mm(0, 3)
        for pr in range(4):
            emit_out_mm(1, pr)
        chain(o_mms[0])
        chain(o_mms[1])

        # ---------------- gate + final mix ----------------
        y_sb = outwork.tile([P, NCH, D], bf16, tag="ysb")
        d1_t = {}
        for c2 in range(NCH):
            # d1 = 0.5*(out - x + bo)  (c1 = 0.5*(bo - x) precomputed on host)
            d1 = outwork.tile([P, D], bf16, tag=f"d1_{c2}")
            nc.vector.scalar_tensor_tensor(
                out=d1, in0=o_ps[c2], scalar=0.5, in1=c1_t[:, c2, :],
                op0=OP.mult, op1=OP.add)
            u_t = outwork.tile([P, D], bf16, tag=f"u_{c2}")
            nc.vector.tensor_add(u_t, d1, xrm[:, c2, :])
            d1_t[c2] = (d1, u_t)

        for c2 in range(NCH):
            d1, u_t = d1_t[c2]
            for half in range(2):
                sl = slice(SQ * half, SQ * half + SQ)
                ps_g = pA.tile([P, SQ], f32, tag="pA")
                g_mms = [nc.tensor.matmul(
                    ps_g, lhsT=xtown[:, dt, P * c2:P * c2 + P],
                    rhs=wgx_t[:, dt, sl],
                    start=(dt == 0), stop=False) for dt in range(4)]
                g_mms += [nc.tensor.matmul(
                    ps_g, lhsT=attn2[:, pr, P * c2:P * c2 + P],
                    rhs=wowg_t[:, pr, sl],
                    start=False, stop=(dbg is None and pr == 3))
                    for pr in range(4)]
                if dbg is not None:
                    g_mms.append(nc.tensor.matmul(
                        ps_g, lhsT=ones1, rhs=bg_row[:, sl],
                        start=False, stop=True))
                chain(g_mms)

                # gate = 0.5*tanh(g/2) + 0.5;  y = x + gate*(out-x)
                #      = x + (tanh(g/2)+1)*d1  with d1 pre-halved
                # tail-latency tuning: the penultimate half's mix goes to
                # Pool so DVE is free for the final half, and the final half
                # is split into two 128-col chunks so the last store waits on
                # as little as possible
                # y = x + (t+1)*d1 = (x + d1) + t*d1 = u + t*d1
                zeng = nc.vector
                t_sb = outwork.tile([P, SQ], bf16, tag=f"t{half}")
                nc.scalar.activation(out=t_sb, in_=ps_g, func=AF.Tanh,
                                     scale=0.5)
                z_sb = outwork.tile([P, SQ], bf16, tag=f"z{half}")
                zeng.tensor_mul(z_sb, t_sb, d1[:, sl])
                nc.vector.tensor_add(y_sb[:, c2, sl], z_sb, u_t[:, sl])
                if c2 == 0 and half == 0:
                    pass  # stored together with half 1 (one HWDGE hold)
                elif c2 == 0:
                    nc.sync.dma_start(out=dy[:, 0, :], in_=y_sb[:, 0, :])
                else:
                    nc.sync.dma_start(out=dy[:, c2, sl], in_=y_sb[:, c2, sl])


def _to_bf16(x):
    import ml_dtypes
    return np.asarray(x, dtype=np.float32).astype(ml_dtypes.bfloat16)


def _to_f8(x):
    import ml_dtypes
    return np.asarray(x, dtype=np.float32).astype(ml_dtypes.float8_e4m3)


def _shard_inputs(inputs, gate_bias):
    x = np.ascontiguousarray(np.asarray(inputs["x"], dtype=np.float32))
    Wq = np.asarray(inputs["Wq"], dtype=np.float32)
    Wk = np.asarray(inputs["Wk"], dtype=np.float32)
    Wv = np.asarray(inputs["Wv"], dtype=np.float32)
    Wo = np.asarray(inputs["Wo"], dtype=np.float32)
    bo = np.asarray(inputs["bo"], dtype=np.float32)
    Wg = np.asarray(inputs["Wg"], dtype=np.float32)
    bg = np.asarray(inputs["bg"], dtype=np.float32)

    def wtile(w):
        # [512, 512] -> [128 p, 4 dt, 512 e] with dt = row//128
        return np.ascontiguousarray(
            _to_bf16(w).reshape(4, P, D).transpose(1, 0, 2))

    def wtile8(w):
        # fp8 with balanced x4 scale (x side carries /4)
        return np.ascontiguousarray(
            _to_f8(4.0 * w).reshape(4, P, D).transpose(1, 0, 2))

    WoWg = Wo @ Wg[D:]
    bg_eff = bg + bo @ Wg[D:]
    www = np.stack([wtile(Wo), wtile(WoWg), wtile(Wg[:D])], axis=1)
    wkvq8 = np.concatenate([wtile8(Wk), wtile8(Wv), wtile8(Wq)],
                           axis=1)  # [P, 12, D]
    shared = {
        "wkvq8": wkvq8,
        "wv16": wtile(Wv),
        "www": np.ascontiguousarray(www),
    }
    if gate_bias:
        shared["bgrow"] = _to_bf16(bg_eff).reshape(1, D)

    in_maps = []
    for cidx in range(NCORE):
        b, j = cidx // 4, cidx % 4
        r0 = SQ * j
        xb = x[b]
        # prefix, zero-padded to 768 rows, chunk-major feature-major layout:
        # xpre[p, c, dt*128+s] = x[b, 128c+s, 128dt+p]
        xpre = np.zeros((PRE, D), np.float32)
        xpre[:r0] = xb[:r0]
        xpre_t = xpre.reshape(NPRE, P, 4, P).transpose(3, 0, 2, 1).reshape(
            P, NPRE, D)
        xo = xb[r0:r0 + SQ]
        # xtown[p, dt, s] = x[b, r0+s, 128dt+p]
        xtown = xo.reshape(SQ, 4, P).transpose(2, 1, 0)
        # row-major own rows: [p, c2, e] = x[b, r0+128c2+p, e]
        xrm = xo.reshape(NCH, P, D).transpose(1, 0, 2)
        c1 = 0.5 * (bo[None, None, :] - xrm)
        m8 = np.zeros((PRE, H), np.float32)
        m8[:r0] = 1.0
        m8_t = m8.reshape(NPRE, P, H).transpose(1, 0, 2)
        xaux = np.stack([xtown.reshape(P, 1024), xrm.reshape(P, 1024),
                         c1.reshape(P, 1024)], axis=1)
        xpre8 = _to_f8(xpre_t / 4.0)
        xtq8 = _to_f8(xtown / 4.0).reshape(P, 2, D)
        f8w = np.concatenate([xpre8[:, 0:1, :], shared["wkvq8"], xtq8],
                             axis=1)
        m = {"xpre": np.ascontiguousarray(xpre8),
             "f8w": np.ascontiguousarray(f8w),
             "xaux": np.ascontiguousarray(_to_bf16(xaux)),
             "m8": np.ascontiguousarray(_to_bf16(m8_t))}
        m.update({k: v for k, v in shared.items() if k != "wkvq8"})
        in_maps.append(m)
    return in_maps


def kernel(**inputs):
    from concourse import bass_utils

    bo = np.asarray(inputs["bo"], dtype=np.float32)
    bg = np.asarray(inputs["bg"], dtype=np.float32)
    Wg = np.asarray(inputs["Wg"], dtype=np.float32)
    gate_bias = bool(np.any(bg + bo @ Wg[D:]))

    nc = _build(gate_bias)
    in_maps = _shard_inputs(inputs, gate_bias)
    trace = os.environ.get("BASS_KERNEL_TRACE", "0") == "1"
    res = bass_utils.run_bass_kernel_spmd(
        nc, in_maps, core_ids=list(range(NCORE)), trace=trace)
    LAST_EXEC_NS[0] = res.exec_time_ns
    x = np.asarray(inputs["x"], dtype=np.float32)
    y = np.empty_like(x)
    for cidx in range(NCORE):
        b, j = cidx // 4, cidx % 4
        yc = np.asarray(res.results[cidx]["y"], dtype=np.float32)
        y[b, SQ * j:SQ * j + SQ] = yc.transpose(1, 0, 2).reshape(SQ, D)
    return y
